# revision 31
# baseline (speedup 1.0000x reference)
"""Trainium2 Bass kernel for nn_CustomNetworkGINSeroMean (GIN message passing +
TopK pooling + SERO readout + BN/FC head).

V3 strategy (data-parallel over batch B=64, 8 graphs per NeuronCore):
  - Dense masked pooling (no gathers); adjacency kept in BOTH orientations
    (adj, adjT) in fp16 so aggregation/augmentation need no runtime transposes.
  - deg folded into the aggregation matmul via an extra "alive" column.
  - Per-node weights never materialized: G = h @ W2r (o-major [din, (o k)]);
    combine sum_k U_k * G_k as one DVE mult (fp16 product) + one DVE k-reduce.
  - Score gets its own matmul columns ([W2k@pw | B2@pw]) so topk is
    independent of the combine; U for all 3 layers precomputed once (pos
    static). All inputs packed host-side into ~12 wide DMAs.
  - Augmentation with diag(alive)-augmented row-masks: one matmul per
    orientation, Z = (B+Md)^T (BT+Md), diag junk stripped by the notI mult.
  - fp16 only where validated flip-free (numpy ablation: 0 topk selection
    flips): adj state, x' state, combine product; h/hT/W2/U stay fp32/f32r.
  - Head: 3-layer-batched SERO in [64, 192] tiles, gelu via erf (shares the
    sigmoid ACT table -> no table reloads), BN rsqrt via bit-trick + Newton
    on DVE, relus on DVE.
  - Pool/GpSimd engine is avoided for bulk tensor work (ucode ops cost
    ~1.2us each and cannot access PSUM).
"""

import numpy as np

import concourse.bass as bass
import concourse.tile as tile
from concourse import bacc, mybir
from concourse.bass_utils import run_bass_kernel_spmd
from concourse.masks import make_identity

F32 = mybir.dt.float32
F32R = mybir.dt.float32r
F16 = mybir.dt.float16
I32 = mybir.dt.int32
AF = mybir.ActivationFunctionType
ALU = mybir.AluOpType
AX = mybir.AxisListType

B, R, D = 64, 100, 100
H = 64
K = 8
N_LAYERS = 3
NCORES = 8
BL = B // NCORES
DIN = (100, 64, 64)
MS = (50, 25, 13)
NEG = -1.0e30
EPS_BN = 1e-5
SQ2I = 0.7071067811865476

TRACE = False
_CACHE = {}


def _emit(tc, io, stage=3):
    import os
    ksub = int(os.environ.get("KSUB", "9"))
    nc = tc.nc
    consts = io["consts_pool"]
    state = io["state_pool"]
    work = io["work_pool"]
    psum = io["psum_pool"]
    psum1 = io["psum1_pool"]
    dram = io["dram_pool"]

    id16 = consts.tile([128, 128], F16, tag="id16")
    nc.sync.dma_start(id16[:], io["id16"][:])
    id32 = consts.tile([128, 128], F32, tag="id32")
    nc.sync.dma_start(id32[:], io["id32"][:])

    def load(name, shape, dt=F32, cast=False):
        t = consts.tile(list(shape), dt, tag=name)
        if cast:
            nc.gpsimd.dma_start(t[:], io[name][:])
        else:
            nc.sync.dma_start(t[:], io[name][:])
        return t

    adT = load("adjT0", (R, BL * R), F16)
    xe = load("xe0", (R, BL * (D + 1)), F16)
    posT = load("posT", (R, BL * R), F32R, cast=True)
    w1c = load("w1c", (R, N_LAYERS * K), F32R, cast=True)
    w2o = load("w2o", (R, N_LAYERS * 512), F32R, cast=True)
    b73 = load("b73", (R, N_LAYERS * 74), F32R, cast=True)
    adj = load("adj0", (R, BL * R), F16)
    notI = load("notI", (R, R), F16)
    ones16 = load("ones16", (R, 1), F16)
    hw = load("hw", (H, 610), F32)
    hc = load("hc", (H, 19), F32)
    magic = consts.tile([H, 1], I32, tag="magic")
    nc.vector.memset(magic[:], 0x5F3759DF)

    def rsqrt(v, FF, C):
        # y = 1/sqrt(v) via bit-trick + 2 Newton iterations (no ACT table)
        sh = work.tile([FF, C], I32, tag="rsq_sh")
        nc.vector.tensor_scalar(sh[:], v.bitcast(I32), 1, None,
                                ALU.logical_shift_right)
        y0i = work.tile([FF, C], I32, tag="rsq_y0")
        nc.vector.tensor_tensor(
            y0i[:], magic[0:FF, 0:1].broadcast_to([FF, C]), sh[:], ALU.subtract
        )
        y = y0i[:].bitcast(F32)
        for _ in range(1):
            t2 = work.tile([FF, C], F32, tag="rsq_t2")
            nc.vector.tensor_tensor(t2[:], y, y, ALU.mult)
            nc.vector.tensor_tensor(t2[:], t2[:], v, ALU.mult)
            nc.vector.tensor_scalar(t2[:], t2[:], -0.5, 1.5, ALU.mult, ALU.add)
            yn = work.tile([FF, C], F32, tag="rsq_y")
            nc.vector.tensor_tensor(yn[:], y, t2[:], ALU.mult)
            y = yn[:]
        return y

    # ---- U = relu(pos @ w1) for all 3 layers at once (posT static) ----
    u_ps = psum.tile([R, 512], F32, tag="U", name="ups")
    for g in range(BL):
        nc.tensor.matmul(
            u_ps[:, g * 24 : (g + 1) * 24],
            posT[:, g * R : (g + 1) * R],
            w1c[:],
        )
    ue = consts.tile([R, BL * N_LAYERS * K], F32, tag="ue")
    nc.vector.tensor_scalar_max(ue[:], u_ps[:, 0 : BL * N_LAYERS * K], 0.0)

    def dump(ap):
        ofin = work.tile([B, 2], F32, tag="ofin")
        nc.vector.memset(ofin[:], 0.0)
        nc.vector.tensor_copy(ofin[0:64, 0:2], ap)
        nc.sync.dma_start(io["out"][:], ofin[:])

    if ksub == 1:
        dump(ue[0:64, 0:2])
        return

    penT = None
    rTs = []
    rloc = dram.tile([N_LAYERS, H, BL], F32, tag="rloc", name="rloc") if stage != 1 else None
    for li in range(N_LAYERS):
        din = DIN[li]
        dp1 = din + 1
        m = MS[li]
        woff = li * 512
        boff = li * 74
        last = li == N_LAYERS - 1
        dn1 = 64 if last else 65  # next-layer xe graph stride

        # ---- y-mms: [A@x | A@alive] per graph ----
        yh = [
            psum1.tile([R, 4 * dp1], F32, tag=f"yh{h}", name=f"yh{h}")
            for h in range(2)
        ]
        for g in range(BL):
            nc.tensor.matmul(
                yh[g // 4][:, (g % 4) * dp1 : (g % 4) * dp1 + dp1],
                adT[:, g * R : (g + 1) * R],
                xe[:, g * dp1 : (g + 1) * dp1],
            )
        invd = work.tile([R, BL], F32, tag="invd")
        dm = work.tile([R, BL], F32, tag="dm")
        for h in range(2):
            dv = yh[h][:].rearrange("p (g c) -> p g c", c=dp1)[:, :, din : din + 1]
            nc.vector.tensor_scalar_max(
                dm[:, h * 4 : h * 4 + 4].unsqueeze(2), dv, 1e-12
            )
        nc.vector.reciprocal(invd[:], dm[:])
        if ksub == 20:
            dump(invd[0:64, 0:2])
            return

        # ---- per-graph: h, hT, G, bias+score ----
        scoreCol = work.tile([R, BL], F32, tag="scoreCol")
        xob = psum1.tile([R, BL * H], F32, tag="xb", name="xb")
        sco = psum1.tile([R, BL * 10], F32, tag="sco", name="sco")
        red_l = []
        for g in range(BL):
            h_t = work.tile([R, din], F32, tag="h")
            nc.vector.scalar_tensor_tensor(
                h_t[:],
                yh[g // 4][:, (g % 4) * dp1 : (g % 4) * dp1 + din],
                invd[:, g : g + 1],
                xe[:, g * dp1 : g * dp1 + din],
                ALU.mult,
                ALU.add,
            )
            tp_t = psum.tile([R, 512], F32, tag="U", name="tp")
            ht_ps = tp_t[0:din, 0:R]
            nc.tensor.transpose(ht_ps, h_t[:], id32[:R, :R])
            hT = work.tile([din, R], F32R, tag="hT")
            nc.scalar.copy(hT[:], ht_ps)
            g_ps = psum.tile([R, 512], F32, tag="G")
            nc.tensor.matmul(g_ps[:], hT[:], w2o[0:din, woff : woff + 512])
            nc.tensor.matmul(
                xob[:, g * H : (g + 1) * H], hT[:], b73[0:din, boff : boff + 64]
            )
            sc_sl = sco[:, g * 10 : (g + 1) * 10]
            nc.tensor.matmul(sc_sl, hT[:], b73[0:din, boff + 64 : boff + 74])
            scr = work.tile([R, 512], F16, tag="pr")
            nc.vector.tensor_tensor(
                scr[:].rearrange("n (o k) -> n o k", k=K),
                g_ps[:].rearrange("n (o k) -> n o k", k=K),
                ue[:, g * 24 + li * K : g * 24 + li * K + K]
                .unsqueeze(1)
                .broadcast_to([R, H, K]),
                ALU.mult,
            )
            red = work.tile([R, H], F16, tag=f"red{g}")
            with nc.allow_low_precision("8-term fp16 sum, validated offline"):
                nc.vector.tensor_reduce(
                    red[:], scr[:].rearrange("n (o k) -> n o k", k=K),
                    AX.X, ALU.add,
                )
            red_l.append(red)

        for g in range(BL):
            scrap = work.tile([R, K], F32, tag="scrap")
            nc.vector.tensor_tensor(
                scrap[:],
                sco[:, g * 10 : g * 10 + 8],
                ue[:, g * 24 + li * K : g * 24 + li * K + K],
                ALU.mult,
            )
            ssum = work.tile([R, 1], F32, tag="ssum")
            nc.vector.tensor_reduce(ssum[:], scrap[:], AX.X, ALU.add)
            nc.vector.tensor_tensor(
                scoreCol[:, g : g + 1],
                ssum[:],
                sco[:, g * 10 + 8 : g * 10 + 9],
                ALU.add,
            )
        if ksub == 2:
            dump(scoreCol[0:64, 0:2])
            return

        # ---- topk (graph-major [BL, R]) ----
        st_t = psum.tile([R, 512], F32, tag="U", name="stp")
        st_ps = st_t[0:BL, 0:R]
        nc.tensor.transpose(st_ps, scoreCol[:], id32[:R, :R])
        sm = work.tile([BL, R], F32, tag="smask")
        if penT is None:
            nc.vector.tensor_copy(sm[:], st_ps)
        else:
            nc.vector.tensor_tensor(sm[:], st_ps, penT[:], ALU.add)
        wk = work.tile([BL, R], F32, tag="wk")
        nc.vector.tensor_copy(wk[:], sm[:])
        for t in range((m + 7) // 8):
            mx = work.tile([BL, 8], F32, tag="mx")
            nc.vector.max(mx[:], wk[:])
            rem = m - 8 * t
            if rem < 8:
                nc.vector.memset(mx[:, rem:8], NEG)
            nc.vector.match_replace(wk[:], mx[:], wk[:], NEG)
        nmT = work.tile([BL, R], F32, tag="nmT")
        nc.vector.tensor_tensor(nmT[:], sm[:], wk[:], ALU.subtract)
        nc.vector.tensor_scalar_min(nmT[:], nmT[:], 1.0)
        penT = work.tile([BL, R], F32, tag=f"penT{li}")
        nc.vector.tensor_scalar(penT[:], nmT[:], 1.0, 1e30, ALU.subtract, ALU.mult)
        sig = work.tile([BL, R], F32, tag="sig")
        nc.scalar.activation(sig[:], sm[:], AF.Sigmoid)
        sclT = work.tile([BL, R], F32, tag="sclT")
        nc.vector.tensor_tensor(sclT[:], sig[:], nmT[:], ALU.mult)
        scl_t = psum.tile([R, 512], F32, tag="U", name="sclp")
        scl_ps = scl_t[0:R, 0:BL]
        nc.tensor.transpose(scl_ps, sclT[:], id32[:BL, :BL])
        scales = work.tile([R, BL], F32, tag="scales")
        nc.vector.tensor_copy(scales[:], scl_ps)

        # ---- next x state (+ alive col), reduces, r ----
        if ksub == 3:
            dump(scales[0:64, 0:2])
            return

        xeN = state.tile([R, BL * dn1], F16, tag=f"xe{li + 1}")
        alc = work.tile([R, BL], F32, tag="alc")
        if not last:
            nc.vector.tensor_scalar(
                xeN[:].rearrange("p (g c) -> p g c", c=dn1)[:, :, 64:65],
                scales[:].unsqueeze(2),
                0.0,
                None,
                ALU.is_gt,
            )
            nc.vector.tensor_scalar(alc[:], scales[:], 0.0, None, ALU.is_gt)
        rt_t = psum.tile([R, 512], F32, tag="U", name="rtp")
        rt_ps = rt_t[0:H, 0:BL]
        for g in range(BL):
            qx = work.tile([R, H], F32, tag="qx")
            nc.vector.tensor_tensor(
                qx[:], red_l[g][:], xob[:, g * H : g * H + 64], ALU.add
            )
            nc.vector.tensor_scalar_mul(
                xeN[:, g * dn1 : g * dn1 + 64], qx[:], scales[:, g : g + 1]
            )
            nc.tensor.matmul(
                rt_ps[:, g : g + 1],
                xeN[:, g * dn1 : g * dn1 + 64],
                ones16[:],
            )
        rT = state.tile([H, BL], F32, tag=f"rT{li}")
        nc.vector.tensor_scalar_mul(rT[:], rt_ps, 1.0 / m)
        rTs.append(rT)
        if rloc is not None:
            nc.sync.dma_start(rloc[li], rT[:])
        if ksub == 4:
            dump(rT[0:64, 0:2])
            return

        # ---- augment adjacency (li < 2) ----
        if not last:
            adjN = state.tile([R, BL * R], F16, tag=f"adj{li + 1}")
            adTN = state.tile([R, BL * R], F16, tag=f"adT{li + 1}")
            for g in range(BL):
                al = alc[:, g : g + 1]
                dal = work.tile([R, R], F16, tag="dal")
                nc.scalar.activation(dal[:], id16[:R, :R], AF.Identity, scale=al)
                bm = work.tile([R, R], F16, tag="bm")
                nc.vector.scalar_tensor_tensor(
                    bm[:], adj[:, g * R : (g + 1) * R], al, dal[:],
                    ALU.mult, ALU.add,
                )
                btm = work.tile([R, R], F16, tag="btm")
                nc.vector.scalar_tensor_tensor(
                    btm[:], adT[:, g * R : (g + 1) * R], al, dal[:],
                    ALU.mult, ALU.add,
                )
                z_t = psum.tile([R, 512], F32, tag="U", name="az")
                z_ps = z_t[0:R, 0:R]
                nc.tensor.matmul(z_ps, btm[:], bm[:])
                nc.vector.tensor_tensor(
                    adjN[:, g * R : (g + 1) * R], z_ps, notI[:], ALU.mult
                )
                zt_t = psum.tile([R, 512], F32, tag="U", name="azt")
                zt_ps = zt_t[0:R, 0:R]
                nc.tensor.matmul(zt_ps, bm[:], btm[:])
                nc.vector.tensor_tensor(
                    adTN[:, g * R : (g + 1) * R], zt_ps, notI[:], ALU.mult
                )
            adj, adT = adjN, adTN
            if ksub == 5:
                d5 = work.tile([R, 2], F32, tag="d5")
                nc.vector.tensor_copy(d5[:], adjN[:, 0:2])
                dump(d5[0:64, 0:2])
                return
        xe = xeN

    if stage == 1:
        # debug: dump rT of all 3 layers, graphs 0-1 -> out[0:64, 0:2]
        ofin = work.tile([B, 2], F32, tag="ofin")
        nc.vector.memset(ofin[:], 0.0)
        nc.vector.tensor_copy(ofin[0:H, 0:2], rTs[2][:, 0:2])
        nc.sync.dma_start(io["out"][:], ofin[:])
        return

    # ---- AllGather r (rloc already streamed per layer) ----
    rg = dram.tile([NCORES, N_LAYERS, H, BL], F32, tag="rgath")
    nc.gpsimd.collective_compute(
        "AllGather",
        ALU.bypass,
        replica_groups=[list(range(NCORES))],
        ins=[rloc[:].opt()],
        outs=[rg[:].opt()],
    )
    rf = state.tile([H, N_LAYERS * B], F32, tag="rf")
    for li in range(N_LAYERS):
        nc.sync.dma_start(
            rf[:, li * B : (li + 1) * B].rearrange("h (c l) -> h c l", c=NCORES),
            rg[:, li].rearrange("c h l -> h c l"),
        )

    # ---- SERO (3 layers batched, feature-major [H, B]) ----
    z_t = psum.tile([R, 512], F32, tag="U", name="hz")
    z_ps = z_t[0:H, 0 : N_LAYERS * B]
    for li in range(N_LAYERS):
        nc.tensor.matmul(
            z_ps[:, li * B : (li + 1) * B],
            hw[:, li * H : (li + 1) * H],
            rf[:, li * B : (li + 1) * B],
        )
    mu3 = work.tile([H, N_LAYERS], F32, tag="mu3")
    nc.vector.tensor_reduce(
        mu3[:], z_ps.rearrange("h (l b) -> h l b", b=B), AX.X, ALU.add
    )
    nc.vector.tensor_scalar_mul(mu3[:], mu3[:], 1.0 / B)
    cen = work.tile([H, N_LAYERS * B], F32, tag="cen")
    nc.vector.tensor_tensor(
        cen[:].rearrange("h (l b) -> h l b", b=B),
        z_ps.rearrange("h (l b) -> h l b", b=B),
        mu3[:].unsqueeze(2).broadcast_to([H, N_LAYERS, B]),
        ALU.subtract,
    )
    sq = work.tile([H, N_LAYERS * B], F32, tag="sq")
    nc.vector.tensor_tensor(sq[:], cen[:], cen[:], ALU.mult)
    var3 = work.tile([H, N_LAYERS], F32, tag="var3")
    nc.vector.tensor_reduce(
        var3[:], sq[:].rearrange("h (l b) -> h l b", b=B), AX.X, ALU.add
    )
    rstd3 = work.tile([H, N_LAYERS], F32, tag="rstd3")
    nc.vector.tensor_scalar(rstd3[:], var3[:], 1.0 / B, EPS_BN, ALU.mult, ALU.add)
    rs3 = rsqrt(rstd3[:], H, N_LAYERS)
    gs = work.tile([H, N_LAYERS], F32, tag="gs")
    nc.vector.tensor_tensor(gs[:], rs3, hc[:, 0:3], ALU.mult)
    zn = work.tile([H, N_LAYERS * B], F32, tag="znf")
    nc.vector.tensor_tensor(
        zn[:].rearrange("h (l b) -> h l b", b=B),
        cen[:].rearrange("h (l b) -> h l b", b=B),
        gs[:].unsqueeze(2).broadcast_to([H, N_LAYERS, B]),
        ALU.mult,
    )
    nc.vector.tensor_tensor(
        zn[:].rearrange("h (l b) -> h l b", b=B),
        zn[:].rearrange("h (l b) -> h l b", b=B),
        hc[:, 6:9].unsqueeze(2).broadcast_to([H, N_LAYERS, B]),
        ALU.add,
    )
    terf = work.tile([H, N_LAYERS * B], F32, tag="terf")
    nc.scalar.activation(terf[:], zn[:], AF.Erf, scale=SQ2I)
    znh = work.tile([H, N_LAYERS * B], F32, tag="znh")
    nc.vector.tensor_scalar_mul(znh[:], zn[:], 0.5)
    e = work.tile([H, N_LAYERS * B], F32, tag="e")
    nc.vector.scalar_tensor_tensor(
        e[:], terf[:], 1.0, znh[:], ALU.add, ALU.mult
    )
    att_t = psum.tile([R, 512], F32, tag="U", name="attp")
    att_ps = att_t[0:H, 0 : N_LAYERS * B]
    for li in range(N_LAYERS):
        nc.tensor.matmul(
            att_ps[:, li * B : (li + 1) * B],
            hw[:, 192 + li * H : 192 + (li + 1) * H],
            e[:, li * B : (li + 1) * B],
        )
    attz = work.tile([H, N_LAYERS * B], F32, tag="attz")
    nc.vector.tensor_tensor(
        attz[:].rearrange("h (l b) -> h l b", b=B),
        att_ps.rearrange("h (l b) -> h l b", b=B),
        hc[:, 9:12].unsqueeze(2).broadcast_to([H, N_LAYERS, B]),
        ALU.add,
    )
    att = work.tile([H, N_LAYERS * B], F32, tag="att")
    nc.scalar.activation(att[:], attz[:], AF.Sigmoid)
    sero = work.tile([H, N_LAYERS * B], F32, tag="sero")
    nc.vector.tensor_tensor(sero[:], rf[:], att[:], ALU.mult)

    # ---- FC head ----
    def bn_feat(z, gcol, bcol, F):
        mu = work.tile([F, 1], F32, tag="bmu")
        nc.vector.tensor_reduce(mu[:], z[:], AX.X, ALU.add)
        nc.vector.tensor_scalar_mul(mu[:], mu[:], 1.0 / B)
        cn = work.tile([F, B], F32, tag="bcen")
        nc.vector.tensor_scalar(cn[:], z[:], mu[:, 0:1], None, ALU.subtract)
        scr0 = work.tile([F, B], F32, tag="bscr")
        v0 = work.tile([F, 1], F32, tag="bv")
        nc.vector.tensor_tensor(scr0[:], cn[:], cn[:], ALU.mult)
        nc.vector.tensor_reduce(v0[:], scr0[:], AX.X, ALU.add)
        nc.vector.tensor_scalar(v0[:], v0[:], 1.0 / B, EPS_BN, ALU.mult, ALU.add)
        rsv = rsqrt(v0[:], F, 1)
        g0 = work.tile([F, 1], F32, tag="bg")
        nc.vector.tensor_tensor(g0[:], rsv, gcol, ALU.mult)
        zn = work.tile([F, B], F32, tag="bzn")
        nc.vector.scalar_tensor_tensor(
            zn[:], cn[:], g0[:, 0:1], bcol.broadcast_to([F, B]), ALU.mult, ALU.add
        )
        return zn

    f1_t = psum.tile([R, 512], F32, tag="U", name="f1p")
    f1_ps = f1_t[0:H, 0:B]
    for li in range(N_LAYERS):
        nc.tensor.matmul(
            f1_ps,
            hw[:, 384 + li * H : 384 + (li + 1) * H],
            sero[:, li * B : (li + 1) * B],
            start=(li == 0),
            stop=(li == N_LAYERS - 1),
        )
    z1 = work.tile([H, B], F32, tag="z1")
    nc.vector.scalar_tensor_tensor(
        z1[:], f1_ps, 0.0, hc[:, 12:13].broadcast_to([H, B]),
        ALU.bypass, ALU.add,
    )
    nc.vector.tensor_scalar_max(z1[:], z1[:], 0.0)
    z1n = bn_feat(z1, hc[:, 13:14], hc[:, 14:15], H)
    f2_t = psum.tile([R, 512], F32, tag="U", name="f2p")
    f2_ps = f2_t[0:32, 0:B]
    nc.tensor.matmul(f2_ps, hw[:, 576:608], z1n[:])
    z2 = work.tile([32, B], F32, tag="z2")
    nc.vector.scalar_tensor_tensor(
        z2[:], f2_ps, 0.0, hc[0:32, 15:16].broadcast_to([32, B]),
        ALU.bypass, ALU.add,
    )
    nc.vector.tensor_scalar_max(z2[:], z2[:], 0.0)
    z2n = bn_feat(z2, hc[0:32, 16:17], hc[0:32, 17:18], 32)
    fo_t = psum.tile([R, 512], F32, tag="U", name="fop")
    fo_ps = fo_t[0:2, 0:B]
    nc.tensor.matmul(fo_ps, hw[0:32, 608:610], z2n[:])
    outT = work.tile([2, B], F32, tag="outT")
    nc.vector.scalar_tensor_tensor(
        outT[:], fo_ps, 0.0, hc[0:2, 18:19].broadcast_to([2, B]),
        ALU.bypass, ALU.add,
    )
    nc.vector.tensor_scalar_max(outT[:], outT[:], 0.0)
    ot_t = psum.tile([R, 512], F32, tag="U", name="otp")
    ot_ps = ot_t[0:B, 0:2]
    nc.tensor.transpose(ot_ps, outT[:], id32[:2, :2])
    ofin = work.tile([B, 2], F32, tag="ofin")
    nc.vector.tensor_copy(ofin[:], ot_ps)
    nc.sync.dma_start(io["out"][:], ofin[:])


def _build(stage=3):
    nc = bacc.Bacc("TRN2", target_bir_lowering=False, debug=False, num_devices=NCORES)
    io = {}

    def dparam(name, shape, dt=F32, kind="ExternalInput"):
        io[name] = nc.dram_tensor(name, list(shape), dt, kind=kind).ap()

    dparam("xe0", (R, BL * (D + 1)), F16)
    dparam("adjT0", (R, BL * R), F16)
    dparam("adj0", (R, BL * R), F16)
    dparam("posT", (R, BL * R))
    dparam("w1c", (R, N_LAYERS * K))
    dparam("w2o", (R, N_LAYERS * 512))
    dparam("b73", (R, N_LAYERS * 74))
    dparam("notI", (R, R), F16)
    dparam("id16", (128, 128), F16)
    dparam("id32", (128, 128), F32)
    dparam("ones16", (R, 1), F16)
    dparam("hw", (H, 610))
    dparam("hc", (H, 19))
    dparam("out", (B, 2), F32, kind="ExternalOutput")

    import contextlib

    with tile.TileContext(nc) as tc:
        with contextlib.ExitStack() as ctx:
            io["consts_pool"] = ctx.enter_context(tc.tile_pool(name="consts", bufs=1))
            io["state_pool"] = ctx.enter_context(tc.tile_pool(name="state", bufs=1))
            io["work_pool"] = ctx.enter_context(tc.tile_pool(name="work", bufs=3))
            io["psum_pool"] = ctx.enter_context(
                tc.tile_pool(name="psum", bufs=2, space="PSUM")
            )
            io["psum1_pool"] = ctx.enter_context(
                tc.tile_pool(name="psum1", bufs=1, space="PSUM")
            )
            io["dram_pool"] = ctx.enter_context(
                tc.tile_pool(name="dram", bufs=1, space="DRAM")
            )
            _emit(tc, io, stage=stage)
    nc.compile()
    return nc


def _prep_shared(inputs):
    f = np.float32
    sh = {}
    sh["notI"] = (1.0 - np.eye(R)).astype(np.float16)
    sh["id16"] = np.eye(128).astype(np.float16)
    sh["id32"] = np.eye(128).astype(np.float32)
    sh["ones16"] = np.ones((R, 1), np.float16)
    sh["w1c"] = np.concatenate(
        [np.asarray(inputs[f"w1_{i}"], f) for i in range(N_LAYERS)], axis=1
    )
    w2o = np.zeros((R, N_LAYERS * 512), f)
    b73 = np.zeros((R, N_LAYERS * 74), f)
    for i in range(N_LAYERS):
        din = DIN[i]
        w2r = np.asarray(inputs[f"w2_{i}"], f).reshape(K, din, H)
        # o-major: [din, (o k)]
        w2o[0:din, i * 512 : (i + 1) * 512] = np.ascontiguousarray(
            w2r.transpose(1, 2, 0).reshape(din, H * K)
        )
        b2r = np.asarray(inputs[f"b2_{i}"], f).reshape(din, H)
        pw = np.asarray(inputs[f"pw_{i}"], f)
        pwn = pw / np.linalg.norm(pw)
        b73[0:din, i * 74 : i * 74 + 64] = b2r
        b73[0:din, i * 74 + 64 : i * 74 + 72] = (w2r @ pwn).T
        b73[0:din, i * 74 + 72] = b2r @ pwn
    sh["w2o"] = w2o
    sh["b73"] = b73
    hw = np.zeros((H, 610), f)
    for i in range(N_LAYERS):
        hw[:, i * H : (i + 1) * H] = np.asarray(inputs[f"sew_{i}"], f)
        hw[:, 192 + i * H : 192 + (i + 1) * H] = np.asarray(inputs[f"saw_{i}"], f)
    # fcw_0 [192, 64] -> chunks [64, 64] per layer (lhsT: contraction on rows)
    fcw0 = np.asarray(inputs["fcw_0"], f).reshape(N_LAYERS, H, H)
    for i in range(N_LAYERS):
        hw[:, 384 + i * H : 384 + (i + 1) * H] = fcw0[i]
    hw[:, 576:608] = np.asarray(inputs["fcw_1"], f)
    hw[0:32, 608:610] = np.asarray(inputs["fw"], f)
    sh["hw"] = hw
    hc = np.zeros((H, 19), f)
    for i in range(N_LAYERS):
        hc[:, i] = np.asarray(inputs[f"sbg_{i}"], f)
        hc[:, 3 + i] = np.asarray(inputs[f"sbb_{i}"], f) * SQ2I
        hc[:, 6 + i] = np.asarray(inputs[f"sbb_{i}"], f)
        hc[:, 9 + i] = np.asarray(inputs[f"sab_{i}"], f)
    hc[:, 12] = np.asarray(inputs["fcb_0"], f)
    hc[:, 13] = np.asarray(inputs["bng_0"], f)
    hc[:, 14] = np.asarray(inputs["bnb_0"], f)
    hc[0:32, 15] = np.asarray(inputs["fcb_1"], f)
    hc[0:32, 16] = np.asarray(inputs["bng_1"], f)
    hc[0:32, 17] = np.asarray(inputs["bnb_1"], f)
    hc[0:2, 18] = np.asarray(inputs["fb"], f)
    sh["hc"] = hc
    return sh


def kernel(**inputs):
    import os

    inputs = {k: np.asarray(v) for k, v in inputs.items()}
    stage = int(os.environ.get("KSTAGE", "3"))
    key = f"nc{stage}"
    if key not in _CACHE:
        _CACHE[key] = _build(stage)
    nc = _CACHE[key]

    sh = _prep_shared(inputs)
    x_f = np.asarray(inputs["x"], np.float32)
    adj_f = np.asarray(inputs["adj"], np.float32)
    pos_f = np.asarray(inputs["pos"], np.float32)
    in_maps = []
    for c in range(NCORES):
        mcore = dict(sh)
        s = slice(c * BL, (c + 1) * BL)
        xg = x_f[s]  # [BL, R, D]
        xe0 = np.ones((BL, R, D + 1), np.float16)
        xe0[:, :, 0:D] = xg.astype(np.float16)
        mcore["xe0"] = np.ascontiguousarray(
            xe0.transpose(1, 0, 2).reshape(R, BL * (D + 1))
        )
        ag = adj_f[s].astype(np.float16)  # [BL, R, R]
        mcore["adj0"] = np.ascontiguousarray(
            ag.transpose(1, 0, 2).reshape(R, BL * R)
        )
        mcore["adjT0"] = np.ascontiguousarray(
            ag.transpose(2, 0, 1).reshape(R, BL * R)
        )
        pg = pos_f[s]
        mcore["posT"] = np.ascontiguousarray(
            pg.transpose(2, 0, 1).reshape(R, BL * R)
        )
        in_maps.append(mcore)

    res = run_bass_kernel_spmd(
        nc, in_maps, core_ids=list(range(NCORES)), trace=TRACE
    )
    _CACHE["last_results"] = res
    return res.results[0]["out"]


# revision 32
# speedup vs baseline: 1.2527x; 1.2527x over previous
"""Trainium2 Bass kernel for nn_CustomNetworkGINSeroMean (GIN message passing +
TopK pooling + SERO readout + BN/FC head).

V3 strategy (data-parallel over batch B=64, 8 graphs per NeuronCore):
  - Dense masked pooling (no gathers); adjacency kept in BOTH orientations
    (adj, adjT) in fp16 so aggregation/augmentation need no runtime transposes.
  - deg folded into the aggregation matmul via an extra "alive" column.
  - Per-node weights never materialized: G = h @ W2r (o-major [din, (o k)]);
    combine sum_k U_k * G_k as one DVE mult (fp16 product) + one DVE k-reduce.
  - Score gets its own matmul columns ([W2k@pw | B2@pw]) so topk is
    independent of the combine; U for all 3 layers precomputed once (pos
    static). All inputs packed host-side into ~12 wide DMAs.
  - Augmentation with diag(alive)-augmented row-masks: one matmul per
    orientation, Z = (B+Md)^T (BT+Md), diag junk stripped by the notI mult.
  - fp16 only where validated flip-free (numpy ablation: 0 topk selection
    flips): adj state, x' state, combine product; h/hT/W2/U stay fp32/f32r.
  - Head: 3-layer-batched SERO in [64, 192] tiles, gelu via erf (shares the
    sigmoid ACT table -> no table reloads), BN rsqrt via bit-trick + Newton
    on DVE, relus on DVE.
  - Pool/GpSimd engine is avoided for bulk tensor work (ucode ops cost
    ~1.2us each and cannot access PSUM).
"""

import numpy as np

import concourse.bass as bass
import concourse.tile as tile
from concourse import bacc, mybir
from concourse.bass_utils import run_bass_kernel_spmd
from concourse.masks import make_identity

F32 = mybir.dt.float32
F32R = mybir.dt.float32r
F16 = mybir.dt.float16
I32 = mybir.dt.int32
AF = mybir.ActivationFunctionType
ALU = mybir.AluOpType
AX = mybir.AxisListType

B, R, D = 64, 100, 100
H = 64
K = 8
N_LAYERS = 3
NCORES = 8
BL = B // NCORES
DIN = (100, 64, 64)
MS = (50, 25, 13)
NEG = -1.0e30
EPS_BN = 1e-5
SQ2I = 0.7071067811865476

TRACE = False
_CACHE = {}


def _emit(tc, io, stage=3):
    import os
    ksub = int(os.environ.get("KSUB", "9"))
    nc = tc.nc
    consts = io["consts_pool"]
    state = io["state_pool"]
    work = io["work_pool"]
    psum = io["psum_pool"]
    psum1 = io["psum1_pool"]
    dram = io["dram_pool"]

    id16 = consts.tile([128, 128], F16, tag="id16")
    nc.sync.dma_start(id16[:], io["id16"][:])
    id32 = consts.tile([128, 128], F32, tag="id32")
    nc.sync.dma_start(id32[:], io["id32"][:])

    def load(name, shape, dt=F32, cast=False):
        t = consts.tile(list(shape), dt, tag=name)
        if cast:
            nc.gpsimd.dma_start(t[:], io[name][:])
        else:
            nc.sync.dma_start(t[:], io[name][:])
        return t

    adT = load("adjT0", (R, BL * R), F16)
    xe = load("xe0", (R, BL * (D + 1)), F16)
    posT = load("posT", (R, BL * R), F32R, cast=True)
    w1c = load("w1c", (R, N_LAYERS * K), F32R, cast=True)
    w2o = load("w2o", (R, N_LAYERS * 512), F32R, cast=True)
    b73 = load("b73", (R, N_LAYERS * 74), F32R, cast=True)
    adj = load("adj0", (R, BL * R), F16)
    notI = load("notI", (R, R), F16)
    ones16 = load("ones16", (R, 1), F16)
    hw = load("hw", (H, 610), F16)
    hc = load("hc", (H, 19), F32)
    magic = consts.tile([H, 1], I32, tag="magic")
    nc.vector.memset(magic[:], 0x5F3759DF)

    def rsqrt(v, FF, C):
        # y = 1/sqrt(v) via bit-trick + 2 Newton iterations (no ACT table)
        sh = work.tile([FF, C], I32, tag="rsq_sh")
        nc.vector.tensor_scalar(sh[:], v.bitcast(I32), 1, None,
                                ALU.logical_shift_right)
        y0i = work.tile([FF, C], I32, tag="rsq_y0")
        nc.vector.tensor_tensor(
            y0i[:], magic[0:FF, 0:1].broadcast_to([FF, C]), sh[:], ALU.subtract
        )
        y = y0i[:].bitcast(F32)
        for _ in range(1):
            t2 = work.tile([FF, C], F32, tag="rsq_t2")
            nc.vector.tensor_tensor(t2[:], y, y, ALU.mult)
            nc.vector.tensor_tensor(t2[:], t2[:], v, ALU.mult)
            nc.vector.tensor_scalar(t2[:], t2[:], -0.5, 1.5, ALU.mult, ALU.add)
            yn = work.tile([FF, C], F32, tag="rsq_y")
            nc.vector.tensor_tensor(yn[:], y, t2[:], ALU.mult)
            y = yn[:]
        return y

    # ---- U = relu(pos @ w1) for all 3 layers at once (posT static) ----
    u_ps = psum.tile([R, 512], F32, tag="U", name="ups")
    for g in range(BL):
        nc.tensor.matmul(
            u_ps[:, g * 24 : (g + 1) * 24],
            posT[:, g * R : (g + 1) * R],
            w1c[:],
        )
    ue = consts.tile([R, BL * N_LAYERS * K], F32, tag="ue")
    nc.vector.tensor_scalar_max(ue[:], u_ps[:, 0 : BL * N_LAYERS * K], 0.0)

    def dump(ap):
        ofin = work.tile([B, 2], F32, tag="ofin")
        nc.vector.memset(ofin[:], 0.0)
        nc.vector.tensor_copy(ofin[0:64, 0:2], ap)
        nc.sync.dma_start(io["out"][:], ofin[:])

    if ksub == 1:
        dump(ue[0:64, 0:2])
        return

    penT = None
    rTs = []
    rloc = dram.tile([N_LAYERS, H, BL], F32, tag="rloc", name="rloc") if stage != 1 else None
    for li in range(N_LAYERS):
        din = DIN[li]
        dp1 = din + 1
        m = MS[li]
        woff = li * 512
        boff = li * 74
        last = li == N_LAYERS - 1
        dn1 = 64 if last else 65  # next-layer xe graph stride

        # ---- y-mms: [A@x | A@alive] per graph ----
        yh = [
            psum1.tile([R, 4 * dp1], F32, tag=f"yh{h}", name=f"yh{h}")
            for h in range(2)
        ]
        for g in range(BL):
            nc.tensor.matmul(
                yh[g // 4][:, (g % 4) * dp1 : (g % 4) * dp1 + dp1],
                adT[:, g * R : (g + 1) * R],
                xe[:, g * dp1 : (g + 1) * dp1],
            )
        invd = work.tile([R, BL], F32, tag="invd")
        dm = work.tile([R, BL], F32, tag="dm")
        for h in range(2):
            dv = yh[h][:].rearrange("p (g c) -> p g c", c=dp1)[:, :, din : din + 1]
            nc.vector.tensor_scalar_max(
                dm[:, h * 4 : h * 4 + 4].unsqueeze(2), dv, 1e-12
            )
        nc.vector.reciprocal(invd[:], dm[:])
        if ksub == 20:
            dump(invd[0:64, 0:2])
            return

        # ---- per-graph: h, hT, G, bias+score ----
        scoreCol = work.tile([R, BL], F32, tag="scoreCol")
        xob = psum1.tile([R, BL * H], F32, tag="xb", name="xb")
        sco = psum1.tile([R, BL * 10], F32, tag="sco", name="sco")
        red_l = []
        for g in range(BL):
            h_t = work.tile([R, din], F32, tag="h")
            nc.vector.scalar_tensor_tensor(
                h_t[:],
                yh[g // 4][:, (g % 4) * dp1 : (g % 4) * dp1 + din],
                invd[:, g : g + 1],
                xe[:, g * dp1 : g * dp1 + din],
                ALU.mult,
                ALU.add,
            )
            tp_t = psum.tile([R, 512], F32, tag="U", name="tp")
            ht_ps = tp_t[0:din, 0:R]
            nc.tensor.transpose(ht_ps, h_t[:], id32[:R, :R])
            hT = work.tile([din, R], F32R, tag="hT")
            nc.scalar.copy(hT[:], ht_ps)
            g_ps = psum.tile([R, 512], F32, tag="G")
            nc.tensor.matmul(g_ps[:], hT[:], w2o[0:din, woff : woff + 512])
            nc.tensor.matmul(
                xob[:, g * H : (g + 1) * H], hT[:], b73[0:din, boff : boff + 64]
            )
            sc_sl = sco[:, g * 10 : (g + 1) * 10]
            nc.tensor.matmul(sc_sl, hT[:], b73[0:din, boff + 64 : boff + 74])
            scr = work.tile([R, 512], F16, tag="pr")
            nc.vector.tensor_tensor(
                scr[:].rearrange("n (o k) -> n o k", k=K),
                g_ps[:].rearrange("n (o k) -> n o k", k=K),
                ue[:, g * 24 + li * K : g * 24 + li * K + K]
                .unsqueeze(1)
                .broadcast_to([R, H, K]),
                ALU.mult,
            )
            red = work.tile([R, H], F16, tag=f"red{g}")
            with nc.allow_low_precision("8-term fp16 sum, validated offline"):
                nc.vector.tensor_reduce(
                    red[:], scr[:].rearrange("n (o k) -> n o k", k=K),
                    AX.X, ALU.add,
                )
            red_l.append(red)

        for g in range(BL):
            scrap = work.tile([R, K], F32, tag="scrap")
            nc.vector.tensor_tensor(
                scrap[:],
                sco[:, g * 10 : g * 10 + 8],
                ue[:, g * 24 + li * K : g * 24 + li * K + K],
                ALU.mult,
            )
            ssum = work.tile([R, 1], F32, tag="ssum")
            nc.vector.tensor_reduce(ssum[:], scrap[:], AX.X, ALU.add)
            nc.vector.tensor_tensor(
                scoreCol[:, g : g + 1],
                ssum[:],
                sco[:, g * 10 + 8 : g * 10 + 9],
                ALU.add,
            )
        if ksub == 2:
            dump(scoreCol[0:64, 0:2])
            return

        # ---- topk (graph-major [BL, R]) ----
        st_t = psum.tile([R, 512], F32, tag="U", name="stp")
        st_ps = st_t[0:BL, 0:R]
        nc.tensor.transpose(st_ps, scoreCol[:], id32[:R, :R])
        sm = work.tile([BL, R], F32, tag="smask")
        if penT is None:
            nc.vector.tensor_copy(sm[:], st_ps)
        else:
            nc.vector.tensor_tensor(sm[:], st_ps, penT[:], ALU.add)
        wk = work.tile([BL, R], F32, tag="wk")
        nc.vector.tensor_copy(wk[:], sm[:])
        for t in range((m + 7) // 8):
            mx = work.tile([BL, 8], F32, tag="mx")
            nc.vector.max(mx[:], wk[:])
            rem = m - 8 * t
            if rem < 8:
                nc.vector.memset(mx[:, rem:8], NEG)
            nc.vector.match_replace(wk[:], mx[:], wk[:], NEG)
        nmT = work.tile([BL, R], F32, tag="nmT")
        nc.vector.tensor_tensor(nmT[:], sm[:], wk[:], ALU.subtract)
        nc.vector.tensor_scalar_min(nmT[:], nmT[:], 1.0)
        penT = work.tile([BL, R], F32, tag=f"penT{li}")
        nc.vector.tensor_scalar(penT[:], nmT[:], 1.0, 1e30, ALU.subtract, ALU.mult)
        sig = work.tile([BL, R], F32, tag="sig")
        nc.scalar.activation(sig[:], sm[:], AF.Sigmoid)
        sclT = work.tile([BL, R], F32, tag="sclT")
        nc.vector.tensor_tensor(sclT[:], sig[:], nmT[:], ALU.mult)
        scl_t = psum.tile([R, 512], F32, tag="U", name="sclp")
        scl_ps = scl_t[0:R, 0:BL]
        nc.tensor.transpose(scl_ps, sclT[:], id32[:BL, :BL])
        scales = work.tile([R, BL], F32, tag="scales")
        nc.vector.tensor_copy(scales[:], scl_ps)

        # ---- next x state (+ alive col), reduces, r ----
        if ksub == 3:
            dump(scales[0:64, 0:2])
            return

        xeN = state.tile([R, BL * dn1], F16, tag=f"xe{li + 1}")
        alc = work.tile([R, BL], F32, tag="alc")
        if not last:
            nc.vector.tensor_scalar(
                xeN[:].rearrange("p (g c) -> p g c", c=dn1)[:, :, 64:65],
                scales[:].unsqueeze(2),
                0.0,
                None,
                ALU.is_gt,
            )
            nc.vector.tensor_scalar(alc[:], scales[:], 0.0, None, ALU.is_gt)
        rt_t = psum.tile([R, 512], F32, tag="U", name="rtp")
        rt_ps = rt_t[0:H, 0:BL]
        for g in range(BL):
            qx = work.tile([R, H], F32, tag="qx")
            nc.vector.tensor_tensor(
                qx[:], red_l[g][:], xob[:, g * H : g * H + 64], ALU.add
            )
            nc.vector.tensor_scalar_mul(
                xeN[:, g * dn1 : g * dn1 + 64], qx[:], scales[:, g : g + 1]
            )
            nc.tensor.matmul(
                rt_ps[:, g : g + 1],
                xeN[:, g * dn1 : g * dn1 + 64],
                ones16[:],
            )
        rT = state.tile([H, BL], F32, tag=f"rT{li}")
        nc.vector.tensor_scalar_mul(rT[:], rt_ps, 1.0 / m)
        rTs.append(rT)
        if rloc is not None:
            nc.sync.dma_start(rloc[li], rT[:])
        if ksub == 4:
            dump(rT[0:64, 0:2])
            return

        # ---- augment adjacency (li < 2) ----
        if not last:
            adjN = state.tile([R, BL * R], F16, tag=f"adj{li + 1}")
            adTN = state.tile([R, BL * R], F16, tag=f"adT{li + 1}")
            for g in range(BL):
                al = alc[:, g : g + 1]
                dal = work.tile([R, R], F16, tag="dal")
                nc.scalar.activation(dal[:], id16[:R, :R], AF.Identity, scale=al)
                bm = work.tile([R, R], F16, tag="bm")
                nc.vector.scalar_tensor_tensor(
                    bm[:], adj[:, g * R : (g + 1) * R], al, dal[:],
                    ALU.mult, ALU.add,
                )
                btm = work.tile([R, R], F16, tag="btm")
                nc.vector.scalar_tensor_tensor(
                    btm[:], adT[:, g * R : (g + 1) * R], al, dal[:],
                    ALU.mult, ALU.add,
                )
                z_t = psum.tile([R, 512], F32, tag="U", name="az")
                z_ps = z_t[0:R, 0:R]
                nc.tensor.matmul(z_ps, btm[:], bm[:])
                nc.vector.tensor_tensor(
                    adjN[:, g * R : (g + 1) * R], z_ps, notI[:], ALU.mult
                )
                zt_t = psum.tile([R, 512], F32, tag="U", name="azt")
                zt_ps = zt_t[0:R, 0:R]
                nc.tensor.matmul(zt_ps, bm[:], btm[:])
                nc.vector.tensor_tensor(
                    adTN[:, g * R : (g + 1) * R], zt_ps, notI[:], ALU.mult
                )
            adj, adT = adjN, adTN
            if ksub == 5:
                d5 = work.tile([R, 2], F32, tag="d5")
                nc.vector.tensor_copy(d5[:], adjN[:, 0:2])
                dump(d5[0:64, 0:2])
                return
        xe = xeN

    if stage == 1:
        # debug: dump rT of all 3 layers, graphs 0-1 -> out[0:64, 0:2]
        ofin = work.tile([B, 2], F32, tag="ofin")
        nc.vector.memset(ofin[:], 0.0)
        nc.vector.tensor_copy(ofin[0:H, 0:2], rTs[2][:, 0:2])
        nc.sync.dma_start(io["out"][:], ofin[:])
        return

    # ---- AllGather r (rloc already streamed per layer) ----
    rg = dram.tile([NCORES, N_LAYERS, H, BL], F32, tag="rgath")
    nc.gpsimd.collective_compute(
        "AllGather",
        ALU.bypass,
        replica_groups=[list(range(NCORES))],
        ins=[rloc[:].opt()],
        outs=[rg[:].opt()],
    )
    rf = state.tile([H, N_LAYERS * B], F16, tag="rf")
    for li in range(N_LAYERS):
        nc.gpsimd.dma_start(
            rf[:, li * B : (li + 1) * B].rearrange("h (c l) -> h c l", c=NCORES),
            rg[:, li].rearrange("c h l -> h c l"),
        )

    # ---- SERO (3 layers batched, feature-major [H, B]) ----
    z_t = psum.tile([R, 512], F32, tag="U", name="hz")
    z_ps = z_t[0:H, 0 : N_LAYERS * B]
    for li in range(N_LAYERS):
        nc.tensor.matmul(
            z_ps[:, li * B : (li + 1) * B],
            hw[:, li * H : (li + 1) * H],
            rf[:, li * B : (li + 1) * B],
        )
    mu3 = work.tile([H, N_LAYERS], F32, tag="mu3")
    nc.vector.tensor_reduce(
        mu3[:], z_ps.rearrange("h (l b) -> h l b", b=B), AX.X, ALU.add
    )
    nc.vector.tensor_scalar_mul(mu3[:], mu3[:], 1.0 / B)
    cen = work.tile([H, N_LAYERS * B], F32, tag="cen")
    nc.vector.tensor_tensor(
        cen[:].rearrange("h (l b) -> h l b", b=B),
        z_ps.rearrange("h (l b) -> h l b", b=B),
        mu3[:].unsqueeze(2).broadcast_to([H, N_LAYERS, B]),
        ALU.subtract,
    )
    sq = work.tile([H, N_LAYERS * B], F32, tag="sq")
    nc.vector.tensor_tensor(sq[:], cen[:], cen[:], ALU.mult)
    var3 = work.tile([H, N_LAYERS], F32, tag="var3")
    nc.vector.tensor_reduce(
        var3[:], sq[:].rearrange("h (l b) -> h l b", b=B), AX.X, ALU.add
    )
    rstd3 = work.tile([H, N_LAYERS], F32, tag="rstd3")
    nc.vector.tensor_scalar(rstd3[:], var3[:], 1.0 / B, EPS_BN, ALU.mult, ALU.add)
    rs3 = rsqrt(rstd3[:], H, N_LAYERS)
    gs = work.tile([H, N_LAYERS], F32, tag="gs")
    nc.vector.tensor_tensor(gs[:], rs3, hc[:, 0:3], ALU.mult)
    zn = work.tile([H, N_LAYERS * B], F32, tag="znf")
    nc.vector.tensor_tensor(
        zn[:].rearrange("h (l b) -> h l b", b=B),
        cen[:].rearrange("h (l b) -> h l b", b=B),
        gs[:].unsqueeze(2).broadcast_to([H, N_LAYERS, B]),
        ALU.mult,
    )
    nc.vector.tensor_tensor(
        zn[:].rearrange("h (l b) -> h l b", b=B),
        zn[:].rearrange("h (l b) -> h l b", b=B),
        hc[:, 6:9].unsqueeze(2).broadcast_to([H, N_LAYERS, B]),
        ALU.add,
    )
    terf = work.tile([H, N_LAYERS * B], F32, tag="terf")
    nc.scalar.activation(terf[:], zn[:], AF.Erf, scale=SQ2I)
    znh = work.tile([H, N_LAYERS * B], F32, tag="znh")
    nc.vector.tensor_scalar_mul(znh[:], zn[:], 0.5)
    e = work.tile([H, N_LAYERS * B], F16, tag="e")
    nc.vector.scalar_tensor_tensor(
        e[:], terf[:], 1.0, znh[:], ALU.add, ALU.mult
    )
    att_t = psum.tile([R, 512], F32, tag="U", name="attp")
    att_ps = att_t[0:H, 0 : N_LAYERS * B]
    for li in range(N_LAYERS):
        nc.tensor.matmul(
            att_ps[:, li * B : (li + 1) * B],
            hw[:, 192 + li * H : 192 + (li + 1) * H],
            e[:, li * B : (li + 1) * B],
        )
    attz = work.tile([H, N_LAYERS * B], F32, tag="attz")
    nc.vector.tensor_tensor(
        attz[:].rearrange("h (l b) -> h l b", b=B),
        att_ps.rearrange("h (l b) -> h l b", b=B),
        hc[:, 9:12].unsqueeze(2).broadcast_to([H, N_LAYERS, B]),
        ALU.add,
    )
    att = work.tile([H, N_LAYERS * B], F32, tag="att")
    nc.scalar.activation(att[:], attz[:], AF.Sigmoid)
    sero = work.tile([H, N_LAYERS * B], F16, tag="sero")
    nc.vector.tensor_tensor(sero[:], rf[:], att[:], ALU.mult)

    # ---- FC head ----
    def bn_feat(z, gcol, bcol, F):
        mu = work.tile([F, 1], F32, tag="bmu")
        nc.vector.tensor_reduce(mu[:], z[:], AX.X, ALU.add)
        nc.vector.tensor_scalar_mul(mu[:], mu[:], 1.0 / B)
        cn = work.tile([F, B], F32, tag="bcen")
        nc.vector.tensor_scalar(cn[:], z[:], mu[:, 0:1], None, ALU.subtract)
        scr0 = work.tile([F, B], F32, tag="bscr")
        v0 = work.tile([F, 1], F32, tag="bv")
        nc.vector.tensor_tensor(scr0[:], cn[:], cn[:], ALU.mult)
        nc.vector.tensor_reduce(v0[:], scr0[:], AX.X, ALU.add)
        nc.vector.tensor_scalar(v0[:], v0[:], 1.0 / B, EPS_BN, ALU.mult, ALU.add)
        rsv = rsqrt(v0[:], F, 1)
        g0 = work.tile([F, 1], F32, tag="bg")
        nc.vector.tensor_tensor(g0[:], rsv, gcol, ALU.mult)
        zn = work.tile([F, B], F16, tag="bzn")
        nc.vector.scalar_tensor_tensor(
            zn[:], cn[:], g0[:, 0:1], bcol.broadcast_to([F, B]), ALU.mult, ALU.add
        )
        return zn

    f1_t = psum.tile([R, 512], F32, tag="U", name="f1p")
    f1_ps = f1_t[0:H, 0:B]
    for li in range(N_LAYERS):
        nc.tensor.matmul(
            f1_ps,
            hw[:, 384 + li * H : 384 + (li + 1) * H],
            sero[:, li * B : (li + 1) * B],
            start=(li == 0),
            stop=(li == N_LAYERS - 1),
        )
    z1 = work.tile([H, B], F32, tag="z1")
    nc.vector.scalar_tensor_tensor(
        z1[:], f1_ps, 0.0, hc[:, 12:13].broadcast_to([H, B]),
        ALU.bypass, ALU.add,
    )
    nc.vector.tensor_scalar_max(z1[:], z1[:], 0.0)
    z1n = bn_feat(z1, hc[:, 13:14], hc[:, 14:15], H)
    f2_t = psum.tile([R, 512], F32, tag="U", name="f2p")
    f2_ps = f2_t[0:32, 0:B]
    nc.tensor.matmul(f2_ps, hw[:, 576:608], z1n[:])
    z2 = work.tile([32, B], F32, tag="z2")
    nc.vector.scalar_tensor_tensor(
        z2[:], f2_ps, 0.0, hc[0:32, 15:16].broadcast_to([32, B]),
        ALU.bypass, ALU.add,
    )
    nc.vector.tensor_scalar_max(z2[:], z2[:], 0.0)
    z2n = bn_feat(z2, hc[0:32, 16:17], hc[0:32, 17:18], 32)
    fo_t = psum.tile([R, 512], F32, tag="U", name="fop")
    fo_ps = fo_t[0:2, 0:B]
    nc.tensor.matmul(fo_ps, hw[0:32, 608:610], z2n[:])
    outT = work.tile([2, B], F32, tag="outT")
    nc.vector.scalar_tensor_tensor(
        outT[:], fo_ps, 0.0, hc[0:2, 18:19].broadcast_to([2, B]),
        ALU.bypass, ALU.add,
    )
    nc.vector.tensor_scalar_max(outT[:], outT[:], 0.0)
    ot_t = psum.tile([R, 512], F32, tag="U", name="otp")
    ot_ps = ot_t[0:B, 0:2]
    nc.tensor.transpose(ot_ps, outT[:], id32[:2, :2])
    ofin = work.tile([B, 2], F32, tag="ofin")
    nc.vector.tensor_copy(ofin[:], ot_ps)
    nc.sync.dma_start(io["out"][:], ofin[:])


def _build(stage=3):
    nc = bacc.Bacc("TRN2", target_bir_lowering=False, debug=False, num_devices=NCORES)
    io = {}

    def dparam(name, shape, dt=F32, kind="ExternalInput"):
        io[name] = nc.dram_tensor(name, list(shape), dt, kind=kind).ap()

    dparam("xe0", (R, BL * (D + 1)), F16)
    dparam("adjT0", (R, BL * R), F16)
    dparam("adj0", (R, BL * R), F16)
    dparam("posT", (R, BL * R))
    dparam("w1c", (R, N_LAYERS * K))
    dparam("w2o", (R, N_LAYERS * 512))
    dparam("b73", (R, N_LAYERS * 74))
    dparam("notI", (R, R), F16)
    dparam("id16", (128, 128), F16)
    dparam("id32", (128, 128), F32)
    dparam("ones16", (R, 1), F16)
    dparam("hw", (H, 610), F16)
    dparam("hc", (H, 19))
    dparam("out", (B, 2), F32, kind="ExternalOutput")

    import contextlib

    with tile.TileContext(nc) as tc:
        with contextlib.ExitStack() as ctx:
            io["consts_pool"] = ctx.enter_context(tc.tile_pool(name="consts", bufs=1))
            io["state_pool"] = ctx.enter_context(tc.tile_pool(name="state", bufs=1))
            io["work_pool"] = ctx.enter_context(tc.tile_pool(name="work", bufs=3))
            io["psum_pool"] = ctx.enter_context(
                tc.tile_pool(name="psum", bufs=2, space="PSUM")
            )
            io["psum1_pool"] = ctx.enter_context(
                tc.tile_pool(name="psum1", bufs=1, space="PSUM")
            )
            io["dram_pool"] = ctx.enter_context(
                tc.tile_pool(name="dram", bufs=1, space="DRAM")
            )
            _emit(tc, io, stage=stage)
    nc.compile()
    return nc


def _prep_shared(inputs):
    f = np.float32
    sh = {}
    sh["notI"] = (1.0 - np.eye(R)).astype(np.float16)
    sh["id16"] = np.eye(128).astype(np.float16)
    sh["id32"] = np.eye(128).astype(np.float32)
    sh["ones16"] = np.ones((R, 1), np.float16)
    sh["w1c"] = np.concatenate(
        [np.asarray(inputs[f"w1_{i}"], f) for i in range(N_LAYERS)], axis=1
    )
    w2o = np.zeros((R, N_LAYERS * 512), f)
    b73 = np.zeros((R, N_LAYERS * 74), f)
    for i in range(N_LAYERS):
        din = DIN[i]
        w2r = np.asarray(inputs[f"w2_{i}"], f).reshape(K, din, H)
        # o-major: [din, (o k)]
        w2o[0:din, i * 512 : (i + 1) * 512] = np.ascontiguousarray(
            w2r.transpose(1, 2, 0).reshape(din, H * K)
        )
        b2r = np.asarray(inputs[f"b2_{i}"], f).reshape(din, H)
        pw = np.asarray(inputs[f"pw_{i}"], f)
        pwn = pw / np.linalg.norm(pw)
        b73[0:din, i * 74 : i * 74 + 64] = b2r
        b73[0:din, i * 74 + 64 : i * 74 + 72] = (w2r @ pwn).T
        b73[0:din, i * 74 + 72] = b2r @ pwn
    sh["w2o"] = w2o
    sh["b73"] = b73
    hw = np.zeros((H, 610), f)
    for i in range(N_LAYERS):
        hw[:, i * H : (i + 1) * H] = np.asarray(inputs[f"sew_{i}"], f)
        hw[:, 192 + i * H : 192 + (i + 1) * H] = np.asarray(inputs[f"saw_{i}"], f)
    # fcw_0 [192, 64] -> chunks [64, 64] per layer (lhsT: contraction on rows)
    fcw0 = np.asarray(inputs["fcw_0"], f).reshape(N_LAYERS, H, H)
    for i in range(N_LAYERS):
        hw[:, 384 + i * H : 384 + (i + 1) * H] = fcw0[i]
    hw[:, 576:608] = np.asarray(inputs["fcw_1"], f)
    hw[0:32, 608:610] = np.asarray(inputs["fw"], f)
    sh["hw"] = hw.astype(np.float16)
    hc = np.zeros((H, 19), f)
    for i in range(N_LAYERS):
        hc[:, i] = np.asarray(inputs[f"sbg_{i}"], f)
        hc[:, 3 + i] = np.asarray(inputs[f"sbb_{i}"], f) * SQ2I
        hc[:, 6 + i] = np.asarray(inputs[f"sbb_{i}"], f)
        hc[:, 9 + i] = np.asarray(inputs[f"sab_{i}"], f)
    hc[:, 12] = np.asarray(inputs["fcb_0"], f)
    hc[:, 13] = np.asarray(inputs["bng_0"], f)
    hc[:, 14] = np.asarray(inputs["bnb_0"], f)
    hc[0:32, 15] = np.asarray(inputs["fcb_1"], f)
    hc[0:32, 16] = np.asarray(inputs["bng_1"], f)
    hc[0:32, 17] = np.asarray(inputs["bnb_1"], f)
    hc[0:2, 18] = np.asarray(inputs["fb"], f)
    sh["hc"] = hc
    return sh


def kernel(**inputs):
    import os

    inputs = {k: np.asarray(v) for k, v in inputs.items()}
    stage = int(os.environ.get("KSTAGE", "3"))
    key = f"nc{stage}"
    if key not in _CACHE:
        _CACHE[key] = _build(stage)
    nc = _CACHE[key]

    sh = _prep_shared(inputs)
    x_f = np.asarray(inputs["x"], np.float32)
    adj_f = np.asarray(inputs["adj"], np.float32)
    pos_f = np.asarray(inputs["pos"], np.float32)
    in_maps = []
    for c in range(NCORES):
        mcore = dict(sh)
        s = slice(c * BL, (c + 1) * BL)
        xg = x_f[s]  # [BL, R, D]
        xe0 = np.ones((BL, R, D + 1), np.float16)
        xe0[:, :, 0:D] = xg.astype(np.float16)
        mcore["xe0"] = np.ascontiguousarray(
            xe0.transpose(1, 0, 2).reshape(R, BL * (D + 1))
        )
        ag = adj_f[s].astype(np.float16)  # [BL, R, R]
        mcore["adj0"] = np.ascontiguousarray(
            ag.transpose(1, 0, 2).reshape(R, BL * R)
        )
        mcore["adjT0"] = np.ascontiguousarray(
            ag.transpose(2, 0, 1).reshape(R, BL * R)
        )
        pg = pos_f[s]
        mcore["posT"] = np.ascontiguousarray(
            pg.transpose(2, 0, 1).reshape(R, BL * R)
        )
        in_maps.append(mcore)

    res = run_bass_kernel_spmd(
        nc, in_maps, core_ids=list(range(NCORES)), trace=TRACE
    )
    _CACHE["last_results"] = res
    return res.results[0]["out"]
